# revision 33
# baseline (speedup 1.0000x reference)
"""Trainium2 Bass kernel for nn_KalmanLSTMPredictor (v3).

Data-parallel across 8 NeuronCores (1024 batch each).  On-chip layout packs 4
batch-groups of 256 on the partition axis: tile [128, 256], partition
p = 32*g + row, column = batch-within-group.

v3 = the proven v1 execution structure (split-mode LSTM, gpsimd m1/hn,
(i,f)/(o,g) psum banks) with three structural upgrades:

1. Encoder Kalman eliminated: P(s)/K(s) are batch-independent, so X(s) is
   linear in the observations z.  The per-step LSTM input projection becomes
   px(s) = Gx_s@zx + Gy_s@zy (constants folded via a ones-row of the Z
   tiles), batch-tanh'd into an XSEQ buffer.  Removes the whole per-step
   Kalman chain (a_enc/ma/mb/mc/mz matmuls + 8 vector ops).

2. predq fusion in the decoder: predq = pred * (pp*s2 + s3) packs pred and
   pred^2 into one operand so  a_x2@pred + q_mm@p2  collapses to AQ@predq
   (and winpAx2/winpQ to winpAQ@predq).

3. Output path: S slices dump to DRAM in chunks during the decoder; the P00
   rows are reloaded packed [128, 480] (DRAM-side rearrange) and sqrt'd
   once.  Startup weight DMAs are split and ordered by first use.
"""

import os
from contextlib import ExitStack

import numpy as np

DT = 0.1
B_SZ = 8192
N_CORES = 8
B_CORE = B_SZ // N_CORES          # 1024
N_GROUPS = 4
FD = B_CORE // N_GROUPS           # 256 columns per tile
T_HIST = 20
N_ENC = T_HIST - 1                # 19
N_DEC = 30
FEAT = 32
NL = 3
F_MAT = np.array([[1.0, DT, DT * DT / 2], [0.0, 1.0, DT], [0.0, 0.0, 1.0]],
                 dtype=np.float64)

# ---------------------------------------------------------------- row layout
PX_ROWS = [0, 8, 9, 10, 11, 12, 13, 14, 15]
PY_ROWS = [1, 16, 17, 18, 19, 20, 21, 22, 23]


def xrow(axis, i):
    return 2 + 2 * i + axis


def prow(axis, i, j):
    return (PX_ROWS if axis == 0 else PY_ROWS)[3 * i + j]


def ref_xp_index(r):
    for i in range(3):
        if r == xrow(0, i):
            return i
        if r == xrow(1, i):
            return 3 + i
    for k in range(9):
        if r == PX_ROWS[k]:
            return 6 + k
        if r == PY_ROWS[k]:
            return 15 + k
    return None


GATES = ["i", "f", "g", "o"]
GATE_SLICE = {"i": slice(0, 32), "f": slice(32, 64), "g": slice(64, 96),
              "o": slice(96, 128)}
# psum column blocks (baseline layout): bank A = [i, f], bank B = [o, g]
GCOL = {"i": 0, "f": 1, "o": 2, "g": 3}


def round_f32r(a):
    """Round fp32 values to fp32r precision (11-bit mantissa, RNE)."""
    a = np.ascontiguousarray(np.asarray(a, np.float32))
    u = a.view(np.uint32).astype(np.uint64)
    bias = ((u >> 12) & 1) + 0x7FF
    u = (u + bias) & ~np.uint64(0xFFF)
    return u.astype(np.uint32).view(np.float32)


# ------------------------------------------------------------------ host prep
def prep_constants(inp):
    f = {k: np.asarray(v, np.float64) for k, v in inp.items()
         if k not in ("hist", "len_pred")}
    R = [float(f["R_x_"][0]) ** 2, float(f["R_y_"][0]) ** 2]
    G_ax = [f["G_x"], f["G_y"]]
    amax = [float(f["max_accel_x"][0]), float(f["max_accel_y"][0])]
    Q = [np.outer(G_ax[a] * amax[a], G_ax[a] * amax[a]) for a in range(2)]
    F = F_MAT
    # torch inits BOTH axes with x stats
    P0 = np.diag([R[0], float(f["velocity_std_x"][0]) ** 2,
                  float(f["acceleration_std_x"][0]) ** 2])

    # host Kalman recursion: P(s), and X(s) = Wz_s @ zvec coefficients
    P = [P0.copy(), P0.copy()]
    Wz = [np.zeros((3, T_HIST)), np.zeros((3, T_HIST))]
    Wz[0][0, 0] = 1.0
    Wz[1][0, 0] = 1.0
    P_hist = [[P[0].copy()], [P[1].copy()]]
    Wz_hist = [[Wz[0].copy()], [Wz[1].copy()]]
    for s in range(1, T_HIST):
        for a in range(2):
            Pp = F @ P[a] @ F.T + Q[a]
            S00 = Pp[0, 0] + R[a]
            K = Pp[:, 0] / S00
            M = (np.eye(3) - np.outer(K, np.eye(3)[0])) @ F
            Wz[a] = M @ Wz[a]
            Wz[a][:, s] += K
            P[a] = Pp - np.outer(K, Pp[0, :])
            P_hist[a].append(P[a].copy())
            Wz_hist[a].append(Wz[a].copy())

    W_in = f["Win_W"]                       # [32, 24] in ref XP order
    winp = np.zeros((32, 24), np.float64)
    for k in range(24):
        winp[:, k] = W_in[:, ref_xp_index(k)]

    # px(s) = Gx_s @ zxvec + Gy_s @ zyvec ; zvec rows 0..19 = z, row 20 = 1.
    # The ones-row of Zx carries c_s (P-part + Win_b).
    Gz = [[], []]
    for s in range(T_HIST):
        c = f["Win_b"].astype(np.float64).copy()
        for a in range(2):
            for i in range(3):
                for j in range(3):
                    c += winp[:, prow(a, i, j)] * P_hist[a][s][i, j]
        for a in range(2):
            Gm = np.zeros((32, 32), np.float64)
            for i in range(3):
                Gm[:, :T_HIST] += np.outer(winp[:, xrow(a, i)],
                                           Wz_hist[a][s][i, :])
            if a == 0:
                Gm[:, T_HIST] = c
            Gz[a].append(Gm)

    # decoder-init S(19): X rows linear in z (fp32), P rows const via the
    # ones-row of the fp32 Zx tile.
    SXz = [np.zeros((32, 32), np.float64), np.zeros((32, 32), np.float64)]
    for a in range(2):
        for i in range(3):
            SXz[a][xrow(a, i), :T_HIST] = Wz_hist[a][T_HIST - 1][i, :]
        for i in range(3):
            for j in range(3):
                SXz[0][prow(a, i, j), T_HIST] = P_hist[a][T_HIST - 1][i, j]

    # decoder state matrices
    A = np.zeros((24, 24), np.float64)
    for a in range(2):
        for i in range(3):
            for j in range(3):
                A[xrow(a, i), xrow(a, j)] = F[i, j]
        for i in range(3):
            for j in range(3):
                for k in range(3):
                    for l in range(3):
                        A[prow(a, i, j), prow(a, k, l)] += F[i, k] * F[j, l]
    A_dec = A.copy()
    A_dec[:, [xrow(0, 2), xrow(1, 2)]] = 0.0
    a_x2 = np.zeros((24, 24))
    for m in range(24):
        a_x2[m, 0] = DT * A[m, xrow(0, 2)]
        a_x2[m, 1] = DT * A[m, xrow(1, 2)]
    oxx = np.outer(f["G_x"], f["G_x"])
    oyy = np.outer(f["G_y"], f["G_y"])
    q_mm = np.zeros((24, 24))
    for i in range(3):
        for j in range(3):
            q_mm[prow(0, i, j), 2] = oxx[i, j]
            q_mm[prow(1, i, j), 3] = oyy[i, j]
    AQ = a_x2 + q_mm                        # disjoint input rows (0,1 / 2,3)
    winpAd = winp @ A_dec
    winpAQ = winp @ AQ
    wout = np.zeros((32, 32))
    wout[:4, :] = f["Wout_W"]

    def bd(mat):
        t32 = np.zeros((32, 32), np.float64)
        t32[:mat.shape[1], :mat.shape[0]] = mat.T
        out = np.zeros((128, 128), np.float32)
        for g in range(N_GROUPS):
            out[32 * g:32 * g + 32, 32 * g:32 * g + 32] = t32
        return out

    def pad2432(m):
        out = np.zeros((32, 32), np.float64)
        out[:m.shape[0], :m.shape[1]] = m
        return out

    wr = {}          # f32r lhsT tiles
    wf = {}          # f32 lhsT tiles (2-pass)
    for s in range(T_HIST):
        wr[f"gx{s}"] = bd(Gz[0][s])
        wr[f"gy{s}"] = bd(Gz[1][s])
    wr["wout"] = bd(wout)
    wr["aq"] = bd(pad2432(AQ))
    wr["winpaq"] = bd(np.ascontiguousarray(winpAQ))
    wf["adec"] = bd(pad2432(A_dec))
    wf["winpad"] = bd(np.ascontiguousarray(winpAd))
    wf["sxx"] = bd(SXz[0])
    wf["sxy"] = bd(SXz[1])
    for pre, wih, whh in (("e", f["enc_Wih"], f["enc_Whh"]),
                          ("d", f["dec_Wih"], f["dec_Whh"])):
        for j in range(NL):
            for G in GATES:
                wr[f"{pre}ih{j}{G}"] = bd(wih[j][GATE_SLICE[G]])
                wr[f"{pre}hh{j}{G}"] = bd(whh[j][GATE_SLICE[G]])
    for k in wr:
        wr[k] = round_f32r(wr[k])

    def bias_tile(v32):
        return np.tile(np.asarray(v32, np.float32).reshape(32, 1),
                       (N_GROUPS, 1))

    bwout = np.zeros(32)
    bwout[:4] = f["Wout_b"]
    s2 = np.zeros(32)
    s2[2:4] = 1.0
    s3 = np.zeros(32)
    s3[0:2] = 1.0
    s3[2:4] = bwout[2:4]
    bwin = np.zeros(32)
    bwin[:] = f["Win_b"]
    biases = {"bwout": bias_tile(bwout), "s2": bias_tile(s2),
              "s3": bias_tile(s3), "bwin": bias_tile(bwin)}
    for pre, bih, bhh in (("e", f["enc_bih"], f["enc_bhh"]),
                          ("d", f["dec_bih"], f["dec_bhh"])):
        for j in range(NL):
            bsum = bih[j] + bhh[j]
            for G in GATES:
                biases[f"b{pre}{j}{G}"] = bias_tile(bsum[GATE_SLICE[G]])
    return wr, wf, biases


def pack_per_core(hist):
    """zr [128, 2*FD] f32r (zx|zy), zf [128, 2*FD] f32; row 32g+u = z(u),
    row 32g+20 = 1.0."""
    hist = np.asarray(hist, np.float32)
    zrs, zfs = [], []
    for c in range(N_CORES):
        h = hist[c * B_CORE:(c + 1) * B_CORE]
        hg = h.reshape(N_GROUPS, FD, T_HIST, 2)
        z = np.zeros((128, 2 * FD), np.float32)
        for g in range(N_GROUPS):
            for a in range(2):
                z[32 * g:32 * g + T_HIST, a * FD:a * FD + FD] = \
                    hg[g, :, :, a].T
            z[32 * g + T_HIST, :] = 1.0
        zfs.append(z)
        zrs.append(round_f32r(z))
    return zrs, zfs


# ------------------------------------------------------- numpy golden model
def golden(inp):
    wr, wf, biases = prep_constants(inp)
    hist = np.asarray(inp["hist"], np.float32)
    B = hist.shape[0]

    def eff(d, name):
        return d[name][:32, :32].T.astype(np.float32)

    bv = {k: v[:32, 0].astype(np.float32) for k, v in biases.items()}
    Zf = [None, None]
    Zrr = [None, None]
    for a in range(2):
        zf = np.zeros((32, B), np.float32)
        zf[:T_HIST] = hist[:, :, a].T
        zf[T_HIST] = 1.0
        Zf[a] = zf
        Zrr[a] = round_f32r(zf)

    def sig(x):
        return 1.0 / (1.0 + np.exp(-x))

    X_t = []
    for s in range(T_HIST):
        px = (eff(wr, f"gx{s}") @ Zrr[0] + eff(wr, f"gy{s}") @ Zrr[1])
        X_t.append(round_f32r(np.tanh(px.astype(np.float32))))

    H = [np.zeros((32, B), np.float32) for _ in range(NL)]
    C = [np.zeros((32, B), np.float32) for _ in range(NL)]

    def lstm_stack(x, pre):
        for j in range(NL):
            g = {}
            for G in GATES:
                g[G] = (eff(wr, f"{pre}ih{j}{G}") @ x
                        + eff(wr, f"{pre}hh{j}{G}") @ H[j]
                        + bv[f"b{pre}{j}{G}"][:, None]).astype(np.float32)
            si, sf, so = sig(g["i"]), sig(g["f"]), sig(g["o"])
            tg = np.tanh(g["g"])
            m1 = round_f32r(sf * C[j])
            m2 = round_f32r(si * tg)
            C[j] = round_f32r(m1 + m2)
            H[j] = round_f32r(so * np.tanh(C[j]))
            x = C[j]
        return x

    for s in range(N_ENC):
        lstm_stack(X_t[s], "e")

    S = (eff(wf, "sxx") @ Zf[0] + eff(wf, "sxy") @ Zf[1]).astype(np.float32)

    A_d = eff(wf, "adec")
    AQ = eff(wr, "aq")
    wAd = eff(wf, "winpad")
    wAQ = eff(wr, "winpaq")
    WOUT = eff(wr, "wout")

    out = np.zeros((B, N_DEC, 5), np.float32)
    x = X_t[19]
    for t in range(N_DEC):
        c2 = lstm_stack(x, "d")
        pp = (WOUT @ c2).astype(np.float32)
        pred = round_f32r(pp + bv["bwout"][:, None])
        t2 = (pp * bv["s2"][:, None] + bv["s3"][:, None]).astype(np.float32)
        predq = round_f32r(pred * t2)
        if t < N_DEC - 1:
            pxn = (wAd @ S + wAQ @ predq).astype(np.float32)
            x = round_f32r(np.tanh(pxn + bv["bwin"][:, None]))
        S = (A_d @ S + AQ @ predq).astype(np.float32)
        out[:, t, 0] = S[2]
        out[:, t, 1] = S[3]
        out[:, t, 2] = np.sqrt(np.maximum(S[0], 0.0))
        out[:, t, 3] = np.sqrt(np.maximum(S[1], 0.0))
    return out


# ------------------------------------------------------------- bass kernel
WR_ORDER = ([f"g{ax}{s}" for s in range(4) for ax in "xy"]
            + [f"e{w}{j}{G}" for j in range(NL) for w in ("ih", "hh")
               for G in GATES]
            + [f"g{ax}{s}" for s in range(4, T_HIST) for ax in "xy"]
            + [f"d{w}{j}{G}" for j in range(NL) for w in ("ih", "hh")
               for G in GATES]
            + ["wout", "aq", "winpaq"])
WF_ORDER = ["adec", "winpad", "sxx", "sxy"]
BIAS_ORDER = (["bwout", "s2", "s3", "bwin"]
              + [f"b{p}{j}{G}" for p in "ed" for j in range(NL)
                 for G in GATES])
WR_CHUNKS = [8, 32, 64, len(WR_ORDER)]    # gx/gy 0-3 | enc | G 4-19 | dec+


def build_nc(n_enc=N_ENC, n_dec=N_DEC, fd=FD):
    import concourse.bacc as bacc
    import concourse.tile as tile
    from concourse import mybir

    AF = mybir.ActivationFunctionType
    f32 = mybir.dt.float32
    f32r = mybir.dt.float32r

    nc = bacc.Bacc("TRN2", target_bir_lowering=False, debug=False,
                   num_devices=N_CORES)

    wrdram = nc.dram_tensor("wpackr", [128, 128 * len(WR_ORDER)], f32r,
                            kind="ExternalInput").ap()
    wfdram = nc.dram_tensor("wpackf", [128, 128 * len(WF_ORDER)], f32,
                            kind="ExternalInput").ap()
    bdram = nc.dram_tensor("bpack", [128, len(BIAS_ORDER)], f32,
                           kind="ExternalInput").ap()
    zrdram = nc.dram_tensor("zr", [128, 2 * fd], f32r,
                            kind="ExternalInput").ap()
    zfdram = nc.dram_tensor("zf", [128, 2 * fd], f32,
                            kind="ExternalInput").ap()
    zerodram = nc.dram_tensor("zeros", [128, fd], f32r,
                              kind="ExternalInput").ap()
    sdump = nc.dram_tensor("sdump", [128, n_dec * fd], f32,
                           kind="ExternalOutput").ap()
    sq_out = nc.dram_tensor("sq_out", [128, n_dec * fd // 16], f32,
                            kind="ExternalOutput").ap()

    NCH = 16
    chw = n_dec * fd // NCH                   # 480

    with tile.TileContext(nc) as tc, ExitStack() as ctx:
        consts = ctx.enter_context(tc.tile_pool(name="consts", bufs=1))
        work = ctx.enter_context(tc.tile_pool(name="work", bufs=2))
        # deep pool for the per-layer gate/mul tiles: with bufs=2 a gate ACT
        # WAR-stalls on the (slow, semaphore-heavy) gpsimd reader from only
        # 2 generations back, serializing the whole pipeline through gpsimd.
        work4 = ctx.enter_context(tc.tile_pool(name="work4", bufs=4))
        carry = ctx.enter_context(tc.tile_pool(name="carry", bufs=2))
        # PSUM (8 banks): pgates [128,4fd] x2 = 4, psum_a {ps, pu, pxb} = 3,
        # psum_b {px} = 1.
        psum_a = ctx.enter_context(
            tc.tile_pool(name="psum_a", bufs=1, space="PSUM"))
        psum_b = ctx.enter_context(
            tc.tile_pool(name="psum_b", bufs=1, space="PSUM"))
        pgates = ctx.enter_context(
            tc.tile_pool(name="pgates", bufs=2, space="PSUM"))

        # ---------------- DMAs (priority order)
        WR = consts.tile([128, 128 * len(WR_ORDER)], f32r)
        ZR = consts.tile([128, 2 * fd], f32r)
        ZF = consts.tile([128, 2 * fd], f32)
        WF = consts.tile([128, 128 * len(WF_ORDER)], f32)
        BIAS = consts.tile([128, len(BIAS_ORDER)], f32)
        nc.sync.dma_start(out=ZR, in_=zrdram)
        lo = 0
        for hi in WR_CHUNKS:
            nc.sync.dma_start(out=WR[:, 128 * lo:128 * hi],
                              in_=wrdram[:, 128 * lo:128 * hi])
            if lo == 0:
                nc.sync.dma_start(out=BIAS, in_=bdram)
            lo = hi
        nc.sync.dma_start(out=ZF, in_=zfdram)
        nc.sync.dma_start(out=WF, in_=wfdram)

        XSEQ = consts.tile([128, T_HIST * fd], f32r)
        SSEQ = consts.tile([128, n_dec * fd], f32)
        PT = consts.tile([128, n_dec * fd // NCH], f32)

        def wre(name):
            i = WR_ORDER.index(name)
            return WR[:, 128 * i:128 * i + 128]

        def wfe(name):
            i = WF_ORDER.index(name)
            return WF[:, 128 * i:128 * i + 128]

        def b(name):
            i = BIAS_ORDER.index(name)
            return BIAS[:, i:i + 1]

        ZX_R = ZR[:, 0:fd]
        ZY_R = ZR[:, fd:2 * fd]

        # encoder px, batches of 2 steps in one psum bank (psum_a "pxb")
        def emit_px_group(grp):
            pxb = psum_a.tile([128, 2 * fd], f32, tag="pxb")
            for k in range(2):
                s = 2 * grp + k
                dst = pxb[:, k * fd:(k + 1) * fd]
                nc.tensor.matmul(dst, wre(f"gx{s}"), ZX_R,
                                 start=(k == 0), stop=False)
                nc.tensor.matmul(dst, wre(f"gy{s}"), ZY_R,
                                 start=False, stop=(k == 1))
            nc.scalar.activation(XSEQ[:, 2 * grp * fd:(2 * grp + 2) * fd],
                                 pxb, AF.Tanh)

        px_emitted = 0
        for grp in range(3):
            emit_px_group(grp)
            px_emitted += 1

        # ---------------- H/C init from the zeros dram (baseline pattern)
        H = []
        C = []
        for j in range(NL):
            h = carry.tile([128, fd], f32r, tag=f"h{j}")
            c = carry.tile([128, fd], f32r, tag=f"c{j}")
            nc.sync.dma_start(out=h, in_=zerodram)
            nc.sync.dma_start(out=c, in_=zerodram)
            H.append(h)
            C.append(c)

        def lstm_step(x0, pre, split=True):
            """3-layer stack.  split: layer j>0 consumes the (m1, m2) pair
            (shorter x-chain, 12 matmuls/layer); non-split: consumes cn
            (8 matmuls/layer — better for the pipelined encoder)."""
            xl = [x0]
            out_pair = None
            for j in range(NL):
                pg = pgates.tile([128, 4 * fd], f32, tag="pg")
                for bank, pair in (("A", ("i", "f")), ("B", ("o", "g"))):
                    for gi, G in enumerate(pair):
                        cols = slice(GCOL[G] * fd, (GCOL[G] + 1) * fd)
                        nc.tensor.matmul(pg[:, cols], wre(f"{pre}hh{j}{G}"),
                                         H[j], start=(gi == 0), stop=False)
                    for xi, xop in enumerate(xl):
                        for gi, G in enumerate(pair):
                            cols = slice(GCOL[G] * fd, (GCOL[G] + 1) * fd)
                            nc.tensor.matmul(
                                pg[:, cols], wre(f"{pre}ih{j}{G}"), xop,
                                start=False,
                                stop=(xi == len(xl) - 1 and gi == 1))
                gact = {}
                for G, fn in (("f", AF.Sigmoid), ("i", AF.Sigmoid),
                              ("g", AF.Tanh), ("o", AF.Sigmoid)):
                    t = work4.tile([128, fd], f32, tag=f"ga{G}")
                    cols = slice(GCOL[G] * fd, (GCOL[G] + 1) * fd)
                    nc.scalar.activation(t, pg[:, cols], fn,
                                         bias=b(f"b{pre}{j}{G}"))
                    gact[G] = t
                m1 = work4.tile([128, fd], f32r, tag="m1r")
                nc.vector.tensor_mul(m1, gact["f"], C[j].bitcast(f32))
                m2 = work4.tile([128, fd], f32r, tag="m2r")
                nc.vector.tensor_mul(m2, gact["i"], gact["g"])
                cn = carry.tile([128, fd], f32r, tag=f"c{j}")
                nc.vector.tensor_add(cn, m1.bitcast(f32), m2.bitcast(f32))
                tc_t = work4.tile([128, fd], f32, tag="tc")
                nc.scalar.activation(tc_t, cn.bitcast(f32), AF.Tanh)
                hn = carry.tile([128, fd], f32r, tag=f"h{j}")
                nc.vector.tensor_mul(hn, gact["o"], tc_t)
                C[j] = cn
                H[j] = hn
                xl = [m1, m2] if split else [cn]
                out_pair = (m1, m2)
            return out_pair

        # ---------------- encoder LSTM over precomputed x(s)
        for t in range(n_enc):
            while px_emitted < T_HIST // 2 and 2 * px_emitted <= t + 4:
                emit_px_group(px_emitted)
                px_emitted += 1
            lstm_step(XSEQ[:, t * fd:(t + 1) * fd], "e", split=False)

        # ---------------- decoder init S via fp32 path
        psi = psum_a.tile([128, 2 * fd], f32, tag="pxb")
        nc.tensor.matmul(psi[:, 0:fd], wfe("sxx"), ZF[:, 0:fd],
                         start=True, stop=False)
        nc.tensor.matmul(psi[:, 0:fd], wfe("sxy"), ZF[:, fd:2 * fd],
                         start=False, stop=True)
        s_cur = work.tile([128, fd], f32, tag="scur")
        nc.vector.tensor_scalar_add(s_cur, psi[:, 0:fd], 0.0)

        # ---------------- decoder
        x_direct = XSEQ[:, n_enc * fd:(n_enc + 1) * fd]
        px = None
        for t in range(n_dec):
            last = t == n_dec - 1
            if t == 0:
                m1c, m2c = lstm_step(x_direct, "d")
            else:
                xn = work.tile([128, fd], f32r, tag="xn")
                nc.scalar.activation(xn, px, AF.Tanh, bias=b("bwin"))
                m1c, m2c = lstm_step(xn, "d")
            pp = psum_a.tile([128, fd], f32, tag="pu")
            nc.tensor.matmul(pp, wre("wout"), m1c, start=True, stop=False)
            nc.tensor.matmul(pp, wre("wout"), m2c, start=False, stop=True)
            pred = work.tile([128, fd], f32r, tag="pred")
            nc.vector.tensor_scalar_add(pred, pp, b("bwout"))
            t2 = work.tile([128, fd], f32, tag="t2")
            nc.vector.tensor_scalar(t2, pp, b("s2"), b("s3"),
                                    op0=mybir.AluOpType.mult,
                                    op1=mybir.AluOpType.add)
            predq = work.tile([128, fd], f32r, tag="predq")
            nc.vector.tensor_mul(predq, pred.bitcast(f32), t2)
            if not last:
                px = psum_b.tile([128, fd], f32, tag="px")
                nc.tensor.matmul(px, wfe("winpad"), s_cur,
                                 start=True, stop=False)
                nc.tensor.matmul(px, wre("winpaq"), predq,
                                 start=False, stop=True)
            ps = psum_a.tile([128, fd], f32, tag="ps")
            nc.tensor.matmul(ps, wfe("adec"), s_cur, start=True, stop=False)
            nc.tensor.matmul(ps, wre("aq"), predq, start=False, stop=True)
            s_new = SSEQ[:, t * fd:(t + 1) * fd]
            nc.vector.tensor_scalar_add(s_new, ps, 0.0)
            s_cur = s_new
            if t % 2 == 1:
                nc.sync.dma_start(out=sdump[:, (t - 1) * fd:(t + 1) * fd],
                                  in_=SSEQ[:, (t - 1) * fd:(t + 1) * fd])
            if t == 17:
                for g in range(N_GROUPS):
                    for a in range(2):
                        q = 2 * g + a
                        nc.sync.dma_start(
                            out=PT[NCH * q:NCH * q + NCH // 2, :],
                            in_=sdump[32 * g + a:32 * g + a + 1,
                                      0:chw * (NCH // 2)].rearrange(
                                "r (k c) -> (r k) c", k=NCH // 2))

        # ---------------- outputs
        for g in range(N_GROUPS):
            for a in range(2):
                q = 2 * g + a
                nc.sync.dma_start(
                    out=PT[NCH * q + NCH // 2:NCH * (q + 1), :],
                    in_=sdump[32 * g + a:32 * g + a + 1,
                              chw * (NCH // 2):].rearrange(
                        "r (k c) -> (r k) c", k=NCH // 2))
        sq = work.tile([128, n_dec * fd // NCH], f32, tag="sq")
        nc.scalar.activation(sq, PT, AF.Sqrt)
        nc.sync.dma_start(out=sq_out, in_=sq)

    nc.compile()
    return nc


_NC_CACHE = {}


def _get_nc():
    key = "full"
    if key not in _NC_CACHE:
        _NC_CACHE[key] = build_nc()
    return _NC_CACHE[key]


def make_in_maps(inputs):
    wr, wf, biases = prep_constants(inputs)
    hist = np.asarray(inputs["hist"], np.float32)
    zrs, zfs = pack_per_core(hist)
    nc = _get_nc()
    wpackr = np.concatenate([wr[n] for n in WR_ORDER], axis=1)
    wpackf = np.concatenate([wf[n] for n in WF_ORDER], axis=1).astype(
        np.float32)
    bpack = np.concatenate([biases[n] for n in BIAS_ORDER],
                           axis=1).astype(np.float32)
    zeros = np.zeros((128, FD), np.float32)
    in_maps = [{"wpackr": wpackr, "wpackf": wpackf, "bpack": bpack,
                "zr": zrs[c], "zf": zfs[c], "zeros": zeros}
               for c in range(N_CORES)]
    return nc, in_maps


def unpack_out(results):
    out = np.zeros((B_SZ, N_DEC, 5), np.float32)
    for c in range(N_CORES):
        r = results[c]
        sd = r["sdump"].reshape(128, N_DEC, FD)
        sq = r["sq_out"].reshape(4, 2, 16, N_DEC * FD // 16)
        sq = sq.reshape(4, 2, N_DEC, FD)
        for g in range(N_GROUPS):
            bsl = slice(c * B_CORE + g * FD, c * B_CORE + (g + 1) * FD)
            out[bsl, :, 0] = sd[32 * g + 2].T
            out[bsl, :, 1] = sd[32 * g + 3].T
            out[bsl, :, 2] = sq[g, 0].T
            out[bsl, :, 3] = sq[g, 1].T
    return out


def kernel(**inputs):
    from concourse.bass_utils import run_bass_kernel_spmd

    assert int(inputs["len_pred"]) == N_DEC
    nc, in_maps = make_in_maps(inputs)
    res = run_bass_kernel_spmd(nc, in_maps, core_ids=list(range(N_CORES)),
                               trace=bool(os.environ.get("KERNEL_TRACE")))
    globals()["_LAST_RESULT"] = res
    return unpack_out(res.results)
